# revision 3
# baseline (speedup 1.0000x reference)
"""Trainium2 Bass kernel for nn_EnhancedSubtractionUnit.

B=8, C=256, H=W=64. Data-parallel over batch: 1 sample per NeuronCore (8 cores).

Per-core pipeline (channel-major layout [C_part, H, W], C split into 2 blocks
of 128 partitions; spatial padded to 66x66 for SAME 3x3 convs):
  1. conv1 (cat(x_low,x_high) 512->256, bf16 hi/lo 3-matmul scheme, BN folded
     into weights; f32r is full-rate but its reduced-precision accumulation
     is too coarse for the offset path) + ReLU via ACT straight into bf16
     h_hi; h_lo via one DVE scalar_tensor_tensor from PSUM
  2. conv2 (256->2 offsets): hi/lo scheme M-packed into [w2h|w2l] (M=4), so
     2 matmuls per tap-chunk instead of 3; grid-scale (x32) folded in
  3. offsets transposed to pixel-partition layout (PE transpose), grid math on
     DVE (exact floor via int-cast + is_gt correction), bilinear weights and
     gather indices
  4. grid_sample: indirect-DMA row gather from a host-prepared transposed
     table with guard rows (adjacent-pixel pairs -> 2 gathers / 4 taps),
     bilinear combine via scalar_tensor_tensor FMA, PE transpose back to
     channel-major, diff = x_low - aligned
  5. three 3x3 convs on diff in float32r (full-rate), SE pooling free via ACT
     accum_out during PSUM->SBUF copies, SE matvecs on PE, fused += xd * s
  6. attention conv in float32r (weights replicated to M=128 so attn is
     computed broadcast), sigmoid, out = attn*diff + x_low
"""
import os
import sys

sys.path.insert(0, "/opt/trn_rl_repo")

import numpy as np
import concourse.bass as bass
import concourse.bacc as bacc
import concourse.tile as tile
from concourse import mybir
from concourse.bass_utils import run_bass_kernel_spmd

F32 = mybir.dt.float32
F32R = mybir.dt.float32r
BF16 = mybir.dt.bfloat16
I32 = mybir.dt.int32
ALU = mybir.AluOpType
ACT = mybir.ActivationFunctionType

B, C, H, W = 8, 256, 64, 64
HW = H * W
PH, PW = H + 2, W + 2  # padded spatial
NCORES = 8
EPS = 1e-5
TBL_ROWS = 4160  # >= 4098 guard-padded gather table rows

_nc_cache = {}


def _load_consts(nc, tc, prm):
    """Allocate + DMA-load all load-once constants. Returns (pool, dict)."""
    pc = tc.alloc_tile_pool(name="const", bufs=1)
    cn = {}

    def cload(name, shape, dt, src):
        t = pc.tile(shape, dt, name=name)
        nc.sync.dma_start(t[:], src)
        cn[name] = t

    cload("w2p_sb", [128, 72], BF16, prm["w2p"][:])
    cload("bxy_sb", [128, 64], F32, prm["bxy"][:])
    cload("id_sb", [128, 128], F32, prm["ident"][:])
    cload("b1_sb", [128, 2], F32, prm["b1"][:])
    cload("saw_sb", [128, 2304], F32R, prm["sawT"][:])
    cload("sab_sb", [128, 1], F32, prm["sab_bc"][:])
    cload("db_sb", [128, 6], F32, prm["db2"][:])
    cload("se1_sb", [128, 384], F32, prm["se1T"][:])
    cload("se2_sb", [64, 768], F32, prm["se2T"][:])
    cload("se1b_sb", [64, 3], F32, prm["se1b2"][:])
    cload("se2b_sb", [128, 6], F32, prm["se2b2"][:])
    return pc, cn


def _emit_body(nc, tc, prm, cn, first_iter=True):
    """Emit one full forward pass. prm: dict of DRAM param handles.

    first_iter=False skips pad-border memsets: tile SBUF addresses are
    identical across For_i iterations, borders are only ever written by
    the memsets, and interiors are fully rewritten each pass.
    """
    phases = set(os.environ.get(
        "KERNEL_PHASES", "conv1,conv2,gather,dw,attn").split(","))
    ctx_pools = []

    def memset0(ap):
        if first_iter:
            nc.gpsimd.memset(ap, 0.0)

    def pool(name, bufs=1, space="SBUF"):
        p = tc.alloc_tile_pool(name=name, bufs=bufs, space=space)
        ctx_pools.append(p)
        return p

    pc = pool("scratch", 1)
    ppsum = pool("ppsum", 2, space="PSUM")
    ptpsum = pool("ptpsum", 2, space="PSUM")
    psmall = pool("psmall", 1, space="PSUM")
    phoff = tc.alloc_tile_pool(name="phoff", bufs=1)     # h_pad, dies after ph3
    pconv1 = tc.alloc_tile_pool(name="pconv1", bufs=1)   # xl/xh/w1, dies after conv1

    w2p_sb = cn["w2p_sb"]
    bxy_sb = cn["bxy_sb"]
    id_sb = cn["id_sb"]
    b1_sb = cn["b1_sb"]
    saw_sb = cn["saw_sb"]
    sab_sb = cn["sab_sb"]
    db_sb = cn["db_sb"]
    se1_sb = cn["se1_sb"]
    se2_sb = cn["se2_sb"]
    se1b_sb = cn["se1b_sb"]
    se2b_sb = cn["se2b_sb"]

    # ---------------- phase 0: stage inputs, split into bf16 hi+lo ----------
    xcat_hi = []
    xcat_lo = []
    for b4, (pname, cio) in enumerate(
            [("xl", 0), ("xl", 1), ("xh", 0), ("xh", 1)]):
        stg = pconv1.tile([128, H, W], F32, name="stg", tag="stg", bufs=2)
        nc.sync.dma_start(stg[:], prm[pname][cio * 128:(cio + 1) * 128, :, :])
        thi = pconv1.tile([128, PH, PW], BF16, name=f"xhi{b4}")
        memset0(thi[:])
        nc.vector.tensor_copy(thi[:, 1:1 + H, 1:1 + W], stg[:])
        tlo = pconv1.tile([128, PH, PW], BF16, name=f"xlo{b4}")
        memset0(tlo[:])
        nc.vector.tensor_sub(tlo[:, 1:1 + H, 1:1 + W], stg[:],
                             thi[:, 1:1 + H, 1:1 + W])
        xcat_hi.append(thi)
        xcat_lo.append(tlo)

    h_hi = []
    h_lo = []
    for co in range(2):
        t = phoff.tile([128, PH, PW], BF16, name=f"h_hi{co}")
        memset0(t[:])
        h_hi.append(t)
        t = phoff.tile([128, PH, PW], BF16, name=f"h_lo{co}")
        memset0(t[:])
        h_lo.append(t)

    # ------------ phase 1: conv1 as bf16 hi/lo pair (3 matmuls/tap) --------
    # prefetch both co blocks' weights up front so co=1 never waits
    w1s = []
    for co in range(2 if "conv1" in phases else 0):
        w1h_sb = pconv1.tile([128, 4608], BF16, name="w1h_sb", tag="w1hco",
                             bufs=2)
        nc.sync.dma_start(w1h_sb[:], prm["w1Th"][:, co * 4608:(co + 1) * 4608])
        w1l_sb = pconv1.tile([128, 4608], BF16, name="w1l_sb", tag="w1lco",
                             bufs=2)
        nc.sync.dma_start(w1l_sb[:], prm["w1Tl"][:, co * 4608:(co + 1) * 4608])
        w1s.append((w1h_sb, w1l_sb))
    for co in range(2 if "conv1" in phases else 0):
        w1h_sb, w1l_sb = w1s[co]
        for nt in range(8):
            ps = ppsum.tile([128, 512], F32, name="c1psum", tag="c1psum")
            first = True
            for t9 in range(9):
                dy, dx = t9 // 3 - 1, t9 % 3 - 1
                for ci in range(4):
                    col = (t9 * 4 + ci) * 128
                    rhs_hi = xcat_hi[ci][:, nt * 8 + 1 + dy:nt * 8 + 9 + dy,
                                         1 + dx:65 + dx]
                    rhs_lo = xcat_lo[ci][:, nt * 8 + 1 + dy:nt * 8 + 9 + dy,
                                         1 + dx:65 + dx]
                    last = (t9 == 8 and ci == 3)
                    nc.tensor.matmul(ps[:], w1h_sb[:, col:col + 128], rhs_hi,
                                     start=first, stop=False)
                    nc.tensor.matmul(ps[:], w1h_sb[:, col:col + 128], rhs_lo,
                                     start=False, stop=False)
                    nc.tensor.matmul(ps[:], w1l_sb[:, col:col + 128], rhs_hi,
                                     start=False, stop=last)
                    first = False
            # h_hi = relu(ps + b1) via ACT (bf16 write rounds);
            # h_lo = max(ps + b1, 0) - h_hi via one DVE op reading PSUM
            hiv = h_hi[co][:, nt * 8 + 1:nt * 8 + 9, 1:65]
            nc.scalar.activation(hiv, ps[:], ACT.Relu,
                                 bias=b1_sb[:, co:co + 1], scale=1.0)
            hstg = pconv1.tile([128, 512], F32, name="hstg", tag="hstg", bufs=2)
            nc.scalar.activation(hstg[:], ps[:], ACT.Relu,
                                 bias=b1_sb[:, co:co + 1], scale=1.0)
            nc.vector.tensor_sub(h_lo[co][:, nt * 8 + 1:nt * 8 + 9, 1:65],
                                 hstg[:], hiv)

    pconv1.release()

    # ------- phase 2: conv2 -> offsets; hi/lo M-packed (2 matmuls/tap) -------
    # psA rows = [w2h*h_hi (2) ; w2l*h_hi (2)], psB = w2h*h_lo. The pair-sum
    # across partitions happens after the PE transpose, in the free dim.
    poff = tc.alloc_tile_pool(name="poff", bufs=1)
    offA_sb = poff.tile([4, HW], F32, name="offA_sb")
    offB_sb = poff.tile([2, HW], F32, name="offB_sb")
    if "conv2" not in phases:
        nc.gpsimd.memset(offA_sb[:], 0.0)
        nc.gpsimd.memset(offB_sb[:], 0.0)
    for nt in range(8 if "conv2" in phases else 0):
        psA = psmall.tile([4, 512], F32, name="c2psA", tag="c2psA")
        psB = psmall.tile([2, 512], F32, name="c2psB", tag="c2psB")
        for t9 in range(9):
            dy, dx = t9 // 3 - 1, t9 % 3 - 1
            for ci in range(2):
                c = t9 * 2 + ci
                rhs_hi = h_hi[ci][:, nt * 8 + 1 + dy:nt * 8 + 9 + dy, 1 + dx:65 + dx]
                rhs_lo = h_lo[ci][:, nt * 8 + 1 + dy:nt * 8 + 9 + dy, 1 + dx:65 + dx]
                first = (c == 0)
                last = (c == 17)
                nc.tensor.matmul(psA[:], w2p_sb[:, c * 4:c * 4 + 4],
                                 rhs_hi, start=first, stop=last)
                nc.tensor.matmul(psB[:], w2p_sb[:, c * 4:c * 4 + 2],
                                 rhs_lo, start=first, stop=last)
        nc.vector.tensor_copy(offA_sb[:, nt * 512:(nt + 1) * 512], psA[:])
        nc.vector.tensor_copy(offB_sb[:, nt * 512:(nt + 1) * 512], psB[:])

    # ---------------- phase 3: transpose offsets + grid math ----------------
    # pixel-partition layout: pixel p = j*128 + i -> tile[i, j], j in [0,32)
    pst6 = ptpsum.tile([128, 32, 6], F32, name="offT_psum", bufs=1)
    for j in range(32):
        nc.tensor.transpose(pst6[:, j, 0:4], offA_sb[:, j * 128:(j + 1) * 128],
                            id_sb[:4, :4])
        nc.tensor.transpose(pst6[:, j, 4:6], offB_sb[:, j * 128:(j + 1) * 128],
                            id_sb[:2, :2])
    ixiy = pc.tile([128, 64], F32, name="ixiy")
    ixiy3 = ixiy[:].rearrange("p (j c) -> p j c", c=2)
    # ix/iy = 32*offset + base (scale folded into w2 on host; bxy holds base)
    nc.vector.tensor_copy(ixiy3, pst6[:, :, 0:2])
    nc.vector.tensor_add(ixiy3, ixiy3, pst6[:, :, 2:4])
    nc.vector.tensor_add(ixiy3, ixiy3, pst6[:, :, 4:6])
    nc.vector.tensor_add(ixiy[:], ixiy[:], bxy_sb[:])
    poff.release()
    phoff.release()
    ix = ixiy[:, 0::2]
    iy = ixiy[:, 1::2]

    G = [128, 32]

    def f32t(name):
        return pc.tile(G, F32, name=name)

    # exact floor via int cast + correction
    xi_i = pc.tile(G, I32, name="xi_i")
    nc.vector.tensor_copy(xi_i[:], ix)
    fx0 = f32t("fx0")
    nc.vector.tensor_copy(fx0[:], xi_i[:])
    corr = f32t("corr")
    nc.vector.tensor_tensor(corr[:], fx0[:], ix, op=ALU.is_gt)
    nc.vector.tensor_sub(fx0[:], fx0[:], corr[:])
    yi_i = pc.tile(G, I32, name="yi_i")
    nc.vector.tensor_copy(yi_i[:], iy)
    fy0 = f32t("fy0")
    nc.vector.tensor_copy(fy0[:], yi_i[:])
    corr2 = f32t("corr2")
    nc.vector.tensor_tensor(corr2[:], fy0[:], iy, op=ALU.is_gt)
    nc.vector.tensor_sub(fy0[:], fy0[:], corr2[:])

    wx = f32t("wx")
    nc.vector.tensor_sub(wx[:], ix, fx0[:])
    wy = f32t("wy")
    nc.vector.tensor_sub(wy[:], iy, fy0[:])

    def valid01(src, name):
        v0a = f32t(name + "_0a")
        nc.vector.tensor_scalar(v0a[:], src[:], 0.0, None, op0=ALU.is_ge)
        v0b = f32t(name + "_0b")
        nc.vector.tensor_scalar(v0b[:], src[:], 63.0, None, op0=ALU.is_le)
        v0 = f32t(name + "_0")
        nc.vector.tensor_mul(v0[:], v0a[:], v0b[:])
        v1a = f32t(name + "_1a")
        nc.vector.tensor_scalar(v1a[:], src[:], -1.0, None, op0=ALU.is_ge)
        v1b = f32t(name + "_1b")
        nc.vector.tensor_scalar(v1b[:], src[:], 62.0, None, op0=ALU.is_le)
        v1 = f32t(name + "_1")
        nc.vector.tensor_mul(v1[:], v1a[:], v1b[:])
        return v0, v1

    vx0, vx1 = valid01(fx0, "vx")
    vy0, vy1 = valid01(fy0, "vy")

    # clamped addresses (+1 guard-row shift folded into xc1)
    xc1 = f32t("xc1")  # clamp(fx0, -1, 64) + 1 == clamp(fx0+1, 0, 65)
    nc.vector.tensor_scalar(xc1[:], fx0[:], -1.0, 64.0, op0=ALU.max, op1=ALU.min)
    nc.vector.tensor_scalar_add(xc1[:], xc1[:], 1.0)
    yc0 = f32t("yc0")
    nc.vector.tensor_scalar(yc0[:], fy0[:], 0.0, 63.0, op0=ALU.max, op1=ALU.min)
    yc1 = f32t("yc1")
    nc.vector.tensor_scalar(yc1[:], fy0[:], 1.0, 0.0, op0=ALU.add, op1=ALU.max)
    nc.vector.tensor_scalar_min(yc1[:], yc1[:], 63.0)

    idxA_f = f32t("idxA_f")
    nc.vector.scalar_tensor_tensor(idxA_f[:], yc0[:], 64.0, xc1[:],
                                   op0=ALU.mult, op1=ALU.add)
    idxB_f = f32t("idxB_f")
    nc.vector.scalar_tensor_tensor(idxB_f[:], yc1[:], 64.0, xc1[:],
                                   op0=ALU.mult, op1=ALU.add)
    idxA = pc.tile(G, I32, name="idxA")
    nc.vector.tensor_copy(idxA[:], idxA_f[:])
    idxB = pc.tile(G, I32, name="idxB")
    nc.vector.tensor_copy(idxB[:], idxB_f[:])

    # bilinear weights, validity folded in
    u = f32t("u")  # (1-wx)*vx0
    nc.vector.tensor_scalar(u[:], wx[:], -1.0, 1.0, op0=ALU.mult, op1=ALU.add)
    nc.vector.tensor_mul(u[:], u[:], vx0[:])
    v = f32t("v")  # (1-wy)*vy0
    nc.vector.tensor_scalar(v[:], wy[:], -1.0, 1.0, op0=ALU.mult, op1=ALU.add)
    nc.vector.tensor_mul(v[:], v[:], vy0[:])
    wxv = f32t("wxv")
    nc.vector.tensor_mul(wxv[:], wx[:], vx1[:])
    wyv = f32t("wyv")
    nc.vector.tensor_mul(wyv[:], wy[:], vy1[:])
    w00 = f32t("w00")
    nc.vector.tensor_mul(w00[:], u[:], v[:])
    w01 = f32t("w01")
    nc.vector.tensor_mul(w01[:], wxv[:], v[:])
    w10 = f32t("w10")
    nc.vector.tensor_mul(w10[:], u[:], wyv[:])
    w11 = f32t("w11")
    nc.vector.tensor_mul(w11[:], wxv[:], wyv[:])

    # ---------------- phase 4: gather + bilinear + diff ----------------
    pdiff = tc.alloc_tile_pool(name="pdiff", bufs=1)
    ctx_pools.append(pdiff)
    pwork = tc.alloc_tile_pool(name="pwork", bufs=2)
    diff_pad = []
    for co in range(2):
        t = pdiff.tile([128, PH, PW], F32R, name=f"diff_pad{co}")
        memset0(t[:].bitcast(F32))
        diff_pad.append(t)

    for j in range(32):
        gA = pwork.tile([128, 512], F32, name="gA", tag="gA")
        gB = pwork.tile([128, 512], F32, name="gB", tag="gB")
        if "gather" in phases:
            nc.gpsimd.indirect_dma_start(
                out=gA[:], out_offset=None, in_=prm["xT2"][:],
                in_offset=bass.IndirectOffsetOnAxis(ap=idxA[:, j:j + 1], axis=0))
            nc.gpsimd.indirect_dma_start(
                out=gB[:], out_offset=None, in_=prm["xT2"][:],
                in_offset=bass.IndirectOffsetOnAxis(ap=idxB[:, j:j + 1], axis=0))
        else:
            nc.sync.dma_start(gA[:], prm["xT2"][j * 64:j * 64 + 128, :])
            nc.sync.dma_start(gB[:], prm["xT2"][j * 64:j * 64 + 128, :])
        acc = pwork.tile([128, 256], F32, name="acc", tag="acc")
        nc.vector.tensor_scalar_mul(acc[:], gA[:, 0:256], w00[:, j:j + 1])
        nc.vector.scalar_tensor_tensor(acc[:], gA[:, 256:512], w01[:, j:j + 1],
                                       acc[:], op0=ALU.mult, op1=ALU.add)
        nc.vector.scalar_tensor_tensor(acc[:], gB[:, 0:256], w10[:, j:j + 1],
                                       acc[:], op0=ALU.mult, op1=ALU.add)
        nc.vector.scalar_tensor_tensor(acc[:], gB[:, 256:512], w11[:, j:j + 1],
                                       acc[:], op0=ALU.mult, op1=ALU.add)
        # transpose [128px, 256ch] back to channel-major, diff = x_low - aligned
        for co in range(2):
            pt = ptpsum.tile([128, 128], F32, name="alT_psum", tag="alT")
            nc.tensor.transpose(pt[:], acc[:, co * 128:(co + 1) * 128], id_sb[:])
            xlw = pwork.tile([128, 2, 64], F32, name="xlw", tag="xlw")
            nc.sync.dma_start(xlw[:], prm["xl"][co * 128:(co + 1) * 128,
                                                2 * j:2 * j + 2, :])
            nc.vector.tensor_sub(
                diff_pad[co][:, 2 * j + 1:2 * j + 3, 1:65],
                xlw[:], pt[:])

    # ---------------- phase 5: 3x dynamic-scale conv branches ----------------
    pwork.release()
    pfused = tc.alloc_tile_pool(name="pfused", bufs=1)
    ctx_pools.append(pfused)
    pdwxd = tc.alloc_tile_pool(name="pdwxd", bufs=1)
    fused_pad = []
    for co in range(2):
        t = pfused.tile([128, PH, PW], F32R, name=f"fused_pad{co}")
        memset0(t[:].bitcast(F32))
        fused_pad.append(t)

    # Software-pipelined: emit branch k+1's matmuls before branch k's SE
    # chain so the tiny SE matvecs (which wait on pooled stats) never stall
    # the PE queue between the big conv groups. xd / pooled tags use bufs=2
    # so branch k's data survives while branch k+1 computes.
    def emit_dw_matmuls(k):
        dwk_sb = pdwxd.tile([128, 4608], F32R, name="dwk_sb", tag="dwk",
                            bufs=2)
        nc.sync.dma_start(dwk_sb[:], prm[f"dwT{k}"][:])
        xd = []
        pooled_parts = []
        for co in range(2):
            xd_t = pdwxd.tile([128, HW], F32, name=f"xd{co}", tag=f"xd{co}",
                              bufs=2)
            xd.append(xd_t)
            pp_t = pc.tile([128, 8], F32, name=f"pooled_parts{co}",
                           tag=f"pooled_parts{co}", bufs=2)
            pooled_parts.append(pp_t)
        for co in range(2):
            for nt in range(8):
                ps = ppsum.tile([128, 512], F32, name="dwpsum", tag="c1psum")
                first = True
                for t9 in range(9):
                    dy, dx = t9 // 3 - 1, t9 % 3 - 1
                    for ci in range(2):
                        col = ((t9 * 2 + ci) * 2 + co) * 128
                        nc.tensor.matmul(
                            ps[:],
                            dwk_sb[:, col:col + 128],
                            diff_pad[ci][:, nt * 8 + 1 + dy:nt * 8 + 9 + dy,
                                         1 + dx:65 + dx],
                            start=first, stop=(t9 == 8 and ci == 1))
                        first = False
                nc.scalar.activation(
                    xd[co][:, nt * 512:(nt + 1) * 512], ps[:],
                    ACT.Identity, bias=db_sb[:, 2 * k + co:2 * k + co + 1],
                    scale=1.0, accum_out=pooled_parts[co][:, nt:nt + 1])
        return xd, pooled_parts

    def emit_se(k, xd, pooled_parts):
        # SE block (tiny matvecs); mean 1/HW folded into se1T on host
        pooled = []
        for co in range(2):
            p_t = pc.tile([128, 1], F32, name=f"pooled{co}", tag=f"pooled{co}")
            nc.vector.reduce_sum(p_t[:], pooled_parts[co][:],
                                 axis=mybir.AxisListType.X)
            pooled.append(p_t)
        pse = psmall.tile([128, 2], F32, name="pse", tag="pse")
        nc.tensor.matmul(pse[0:64, 0:1], se1_sb[:, k * 128:k * 128 + 64],
                         pooled[0][:], start=True, stop=False)
        nc.tensor.matmul(pse[0:64, 0:1], se1_sb[:, k * 128 + 64:k * 128 + 128],
                         pooled[1][:], start=False, stop=True)
        h1 = pc.tile([64, 1], F32, name="h1", tag="h1")
        nc.scalar.activation(h1[:], pse[0:64, 0:1], ACT.Relu,
                             bias=se1b_sb[:, k:k + 1], scale=1.0)
        for co in range(2):
            nc.tensor.matmul(pse[:, 1:2],
                             se2_sb[:, (k * 2 + co) * 128:(k * 2 + co + 1) * 128],
                             h1[:], start=True, stop=True)
            s_t = pc.tile([128, 1], F32, name=f"s{co}", tag=f"s{co}")
            nc.scalar.activation(s_t[:], pse[:, 1:2], ACT.Sigmoid,
                                 bias=se2b_sb[:, 2 * k + co:2 * k + co + 1], scale=1.0)
            # fused += xd * s
            if k == 0:
                nc.vector.tensor_scalar_mul(
                    fused_pad[co][:, 1:1 + H, 1:1 + W],
                    xd[co][:].rearrange("p (h w) -> p h w", h=H), s_t[:])
            else:
                nc.vector.scalar_tensor_tensor(
                    fused_pad[co][:, 1:1 + H, 1:1 + W],
                    xd[co][:].rearrange("p (h w) -> p h w", h=H), s_t[:],
                    fused_pad[co][:, 1:1 + H, 1:1 + W].bitcast(F32),
                    op0=ALU.mult, op1=ALU.add)

    if "dw" in phases:
        prev = None
        for k in range(3):
            cur = emit_dw_matmuls(k)
            if prev is not None:
                emit_se(k - 1, *prev)
            prev = cur
        emit_se(2, *prev)

    pdwxd.release()
    pwork = tc.alloc_tile_pool(name="pfinal", bufs=2)
    ctx_pools.append(pwork)

    # ---------------- phase 6: attention + final ----------------
    for nt in range(8):
        attn = pwork.tile([128, 512], F32, name="attn", tag="attn")
        if "attn" in phases:
            ps = ppsum.tile([128, 512], F32, name="sapsum", tag="c1psum")
            first = True
            for t9 in range(9):
                dy, dx = t9 // 3 - 1, t9 % 3 - 1
                for ci in range(2):
                    col = (t9 * 2 + ci) * 128
                    nc.tensor.matmul(
                        ps[:],
                        saw_sb[:, col:col + 128],
                        fused_pad[ci][:, nt * 8 + 1 + dy:nt * 8 + 9 + dy,
                                      1 + dx:65 + dx],
                        start=first, stop=(t9 == 8 and ci == 1))
                    first = False
            nc.scalar.activation(attn[:], ps[:], ACT.Sigmoid, bias=sab_sb[:, 0:1],
                                 scale=1.0)
        else:
            nc.gpsimd.memset(attn[:], 0.5)
        for co in range(2):
            xlt = pwork.tile([128, 512], F32, name="xlt", tag="xlt")
            nc.sync.dma_start(
                xlt[:], prm["xl"][co * 128:(co + 1) * 128, nt * 8:(nt + 1) * 8, :])
            ot = pwork.tile([128, 512], F32, name="ot", tag="ot")
            nc.vector.tensor_mul(
                ot[:], attn[:],
                diff_pad[co][:, nt * 8 + 1:nt * 8 + 9, 1:65].bitcast(F32))
            nc.vector.tensor_add(ot[:], ot[:], xlt[:])
            nc.sync.dma_start(
                prm["out"][co * 128:(co + 1) * 128, nt * 512:(nt + 1) * 512], ot[:])

    for p in reversed(ctx_pools):
        p.release()


def _build(repeat):
    nc = bacc.Bacc()
    prm = {}

    def din(name, shape, dt=F32):
        prm[name] = nc.declare_dram_parameter(name, list(shape), dt, isOutput=False)

    din("xl", [C, H, W])
    din("xh", [C, H, W])
    din("xT2", [TBL_ROWS, 512])
    din("w1Th", [128, 9216], BF16)
    din("w1Tl", [128, 9216], BF16)
    din("b1", [128, 2])
    din("w2p", [128, 72], BF16)
    din("bxy", [128, 64])
    din("ident", [128, 128])
    for k in range(3):
        din(f"dwT{k}", [128, 4608], F32R)
    din("db2", [128, 6])
    din("se1T", [128, 384])
    din("se1b2", [64, 3])
    din("se2T", [64, 768])
    din("se2b2", [128, 6])
    din("sawT", [128, 2304], F32R)
    din("sab_bc", [128, 1])
    prm["out"] = nc.declare_dram_parameter("out", [C, HW], F32, isOutput=True)

    with tile.TileContext(nc) as tc:
        pc_const, cn = _load_consts(nc, tc, prm)
        _emit_body(nc, tc, prm, cn, first_iter=True)
        if repeat > 1:
            with tc.For_i(0, repeat - 1, 1):
                _emit_body(nc, tc, prm, cn, first_iter=False)
        pc_const.release()
    nc.finalize()
    return nc


def _prep_inputs(x_low, x_high, a1w, a1b, bn_g, bn_b, bn_m, bn_v, a2w, a2b,
                 dw, db, se1w, se1b, se2w, se2b, saw, sab):
    """Host-side weight prep shared by all cores + per-core activation prep."""
    import ml_dtypes
    f32 = np.float32
    # conv1 with BN folded
    scale = (bn_g / np.sqrt(bn_v + EPS)).astype(f32)  # [256]
    w1f = (a1w * scale[:, None, None, None]).astype(f32)  # [256,512,3,3]
    b1f = ((a1b - bn_m) * scale + bn_b).astype(f32)  # [256]
    # host lhsT layout [k(128), ty,tx, ci(4), co(2), m(128)] -> [128, 9216]
    arr = w1f.reshape(2, 128, 4, 128, 3, 3)  # [co, m, ci, k, ty, tx]
    w1T = np.ascontiguousarray(arr.transpose(3, 0, 4, 5, 2, 1)).reshape(128, 9216)
    w1Th = w1T.astype(ml_dtypes.bfloat16)
    w1Tl = (w1T - w1Th.astype(np.float32)).astype(ml_dtypes.bfloat16)
    b1h = np.ascontiguousarray(b1f.reshape(2, 128).T)  # [128, 2]

    # conv2, grid scale W/2 = 32 folded in; hi/lo M-packed [w2h|w2l] per chunk
    w2f = (a2w * 32.0).astype(f32)  # [2, 256, 3, 3]
    arr = w2f.reshape(2, 2, 128, 3, 3)  # [m, ci, k, ty, tx]
    w2T = np.ascontiguousarray(arr.transpose(2, 3, 4, 1, 0))  # [k,ty,tx,ci,m]
    w2h = w2T.astype(ml_dtypes.bfloat16)
    w2l = (w2T - w2h.astype(np.float32)).astype(ml_dtypes.bfloat16)
    w2p = np.concatenate([w2h.reshape(128, 18, 2), w2l.reshape(128, 18, 2)],
                         axis=2)  # [k, chunk, 4]
    w2p = np.ascontiguousarray(w2p).reshape(128, 72)

    # base grid (+a2b*32): pixel p = j*128+i ; h=p//64, w=p%64
    lin = np.linspace(-1.0, 1.0, 64, dtype=f32)
    pidx = (np.arange(32)[None, :] * 128 + np.arange(128)[:, None])  # [128,32]
    bx = ((lin[pidx // 64] + 1.0) * 32.0 - 0.5 + 32.0 * f32(a2b[0])).astype(f32)
    by = ((lin[pidx % 64] + 1.0) * 32.0 - 0.5 + 32.0 * f32(a2b[1])).astype(f32)
    bxy = np.empty((128, 64), f32)
    bxy[:, 0::2] = bx
    bxy[:, 1::2] = by

    # diff convs
    dwT = []
    for k in range(3):
        arr = dw[k].astype(f32).reshape(2, 128, 2, 128, 3, 3)  # [co,m,ci,kk,ty,tx]
        dwT.append(np.ascontiguousarray(arr.transpose(3, 4, 5, 2, 0, 1)).reshape(128, 4608))
    db2 = np.ascontiguousarray(db.astype(f32).reshape(3, 2, 128).transpose(2, 0, 1)
                               ).reshape(128, 6)

    # SE (mean 1/HW folded into se1T)
    se1T = np.ascontiguousarray(
        (se1w.astype(f32) / HW).transpose(0, 2, 1).reshape(3, 2, 128, 64)
        .transpose(2, 0, 1, 3)).reshape(128, 384)
    se1b2 = np.ascontiguousarray(se1b.astype(f32).T)  # [64, 3]
    se2T = np.ascontiguousarray(
        se2w.astype(f32).transpose(0, 2, 1).reshape(3, 64, 2, 128)
        .transpose(1, 0, 2, 3)).reshape(64, 768)
    se2b2 = np.ascontiguousarray(se2b.astype(f32).reshape(3, 2, 128)
                                 .transpose(2, 0, 1)).reshape(128, 6)

    # attention conv, weights replicated to M=128
    arr = saw.astype(f32).reshape(1, 2, 128, 3, 3)  # [m=1, ci, k, ty, tx]
    arr = np.broadcast_to(arr, (128, 2, 128, 3, 3))  # replicate m
    sawT = np.ascontiguousarray(arr.transpose(2, 3, 4, 1, 0)).reshape(128, 2304)
    sab_bc = np.full((128, 1), f32(sab[0]), f32)

    shared = dict(w1Th=w1Th, w1Tl=w1Tl, b1=b1h, w2p=w2p, bxy=bxy,
                  ident=np.eye(128, dtype=f32),
                  dwT0=dwT[0], dwT1=dwT[1], dwT2=dwT[2], db2=db2,
                  se1T=se1T, se1b2=se1b2, se2T=se2T, se2b2=se2b2,
                  sawT=sawT, sab_bc=sab_bc)

    in_maps = []
    for b in range(B):
        xlb = np.ascontiguousarray(x_low[b].astype(f32))
        xhb = np.ascontiguousarray(x_high[b].astype(f32))
        XT = np.ascontiguousarray(xhb.reshape(C, HW).T)  # [4096, 256]
        XT2 = np.zeros((TBL_ROWS, 512), f32)
        XT2[1:1 + HW, :256] = XT
        XT2[0:HW, 256:] = XT
        m = dict(shared)
        m["xl"] = xlb
        m["xh"] = xhb
        m["xT2"] = XT2
        in_maps.append(m)
    return in_maps


_last_results = None


def kernel(**inputs):
    global _last_results
    repeat = int(os.environ.get("KERNEL_REPEAT", "1"))
    if repeat not in _nc_cache:
        _nc_cache[repeat] = _build(repeat)
    nc = _nc_cache[repeat]
    in_maps = _prep_inputs(**inputs)
    res = run_bass_kernel_spmd(nc, in_maps, list(range(NCORES)))
    _last_results = res
    out = np.stack([res.results[b]["out"].reshape(C, H, W) for b in range(B)])
    return out.astype(np.float32)


if __name__ == "__main__":
    import reference
    inputs = {k: np.asarray(v) for k, v in reference.setup_inputs().items()}
    expected = np.asarray(reference.reference(**inputs))
    actual = kernel(**inputs)
    err = np.abs(actual - expected).max()
    rel = err / np.abs(expected).max()
    print(f"abs err: {err:.4e}  rel err: {rel:.4e}")



# revision 17
# speedup vs baseline: 33.8214x; 33.8214x over previous
"""Trainium2 Bass kernel for nn_EnhancedSubtractionUnit.

B=8, C=256, H=W=64. Data-parallel over batch: 1 sample per NeuronCore (8 cores).

Per-core pipeline (channel-major layout [C_part, H, W], C split into 2 blocks
of 128 partitions; spatial padded to 66x66 for SAME 3x3 convs):

Front end is software-pipelined by 8-row strip (nt) so the offset/gather
chain hides entirely under conv1/conv2's PE stream:
    stage nt:   conv1(nt)   512->256 bf16 hi/lo 3-matmul scheme (BN folded);
                            f32r/bf16-only schemes are too coarse for the
                            offset path (verified: 2mm -> rel err 0.3)
    stage nt-1: conv2(nt-1) 256->2 offsets, hi/lo M-packed (2 matmuls/chunk),
                            grid scale (x32) folded into weights
    stage nt-2: grid(nt-2)  PE-transpose offsets to pixel-partition layout,
                            exact floor + validity + bilinear weights + gather
                            indices on DVE, issue indirect-DMA row gathers
    stage nt-3: diff(nt-3)  bilinear combine (bf16 gather table, f32 acc),
                            PE-transpose back to channel-major,
                            diff = x_low - aligned (stored bf16)
Back end (unchanged structure, bf16 storage throughout):
    3x DynamicScaleConv branches on diff (bf16 matmuls), SE pooling free via
    ACT accum_out, SE matvecs on PE, fused += xd * s; attention conv (bf16,
    weights replicated to M=128), sigmoid, out = attn*diff + x_low.
"""
import os
import sys

sys.path.insert(0, "/opt/trn_rl_repo")

import numpy as np
import concourse.bass as bass
import concourse.bacc as bacc
import concourse.tile as tile
from concourse import mybir
from concourse.bass_utils import run_bass_kernel_spmd

F32 = mybir.dt.float32
BF16 = mybir.dt.bfloat16
I32 = mybir.dt.int32
ALU = mybir.AluOpType
ACT = mybir.ActivationFunctionType

B, C, H, W = 8, 256, 64, 64
HW = H * W
PH, PW = H + 2, W + 2  # padded spatial
NCORES = 8
EPS = 1e-5
TBL_ROWS = 4160  # >= 4098 guard-padded gather table rows

_nc_cache = {}


def _load_consts(nc, tc, prm):
    """Allocate + DMA-load all load-once constants. Returns (pool, dict)."""
    pc = tc.alloc_tile_pool(name="const", bufs=1)
    cn = {}

    def cload(name, shape, dt, src):
        t = pc.tile(shape, dt, name=name)
        nc.sync.dma_start(t[:], src)
        cn[name] = t

    cload("w2p_sb", [128, 72], BF16, prm["w2p"][:])
    cload("bxy_sb", [128, 64], F32, prm["bxy"][:])
    cload("id_sb", [128, 128], F32, prm["ident"][:])
    cload("b1_sb", [128, 2], F32, prm["b1"][:])
    cload("saw_sb", [128, 2304], BF16, prm["sawT"][:])
    cload("sab_sb", [128, 1], F32, prm["sab_bc"][:])
    cload("db_sb", [128, 6], F32, prm["db2"][:])
    cload("se1_sb", [128, 384], F32, prm["se1T"][:])
    cload("se2_sb", [64, 768], F32, prm["se2T"][:])
    cload("se1b_sb", [64, 3], F32, prm["se1b2"][:])
    cload("se2b_sb", [128, 6], F32, prm["se2b2"][:])
    return pc, cn


def _emit_body(nc, tc, prm, cn, first_iter=True):
    """Emit one full forward pass. prm: dict of DRAM param handles.

    first_iter=False skips pad-border memsets: tile SBUF addresses are
    identical across For_i iterations, borders are only ever written by
    the memsets, and interiors are fully rewritten each pass.
    """
    ctx_pools = []

    def memset0(ap):
        if first_iter:
            nc.gpsimd.memset(ap, 0.0)

    def pool(name, bufs=1, space="SBUF"):
        p = tc.alloc_tile_pool(name=name, bufs=bufs, space=space)
        ctx_pools.append(p)
        return p

    pc = pool("scratch", 1)
    ppsum = pool("ppsum", 2, space="PSUM")
    ptpsum = pool("ptpsum", 2, space="PSUM")
    psmall = pool("psmall", 1, space="PSUM")
    # diff_pad + gather work tiles outlive the conv-era transient pool
    # (the pipeline drain overlaps the first dw-conv groups) -> allocate below
    pdiff = pool("pdiff", 1)
    pgather = pool("pgather", 1)
    ptrans = tc.alloc_tile_pool(name="ptrans", bufs=1)  # dies after conv2(7)

    w2p_sb = cn["w2p_sb"]
    bxy_sb = cn["bxy_sb"]
    id_sb = cn["id_sb"]
    b1_sb = cn["b1_sb"]
    saw_sb = cn["saw_sb"]
    sab_sb = cn["sab_sb"]
    db_sb = cn["db_sb"]
    se1_sb = cn["se1_sb"]
    se2_sb = cn["se2_sb"]
    se1b_sb = cn["se1b_sb"]
    se2b_sb = cn["se2b_sb"]

    diff_pad = []
    for co in range(2):
        t = pdiff.tile([128, PH, PW], BF16, name=f"diff_pad{co}")
        memset0(t[:])
        diff_pad.append(t)

    # ---------------- conv-era tiles ----------------
    # padded bf16 hi/lo inputs, split on host: order [xl0, xl1, xh0, xh1]
    xcat_hi = []
    xcat_lo = []
    for b4, (pname, cio) in enumerate(
            [("xl", 0), ("xl", 1), ("xh", 0), ("xh", 1)]):
        thi = ptrans.tile([128, PH, PW], BF16, name=f"xhi{b4}")
        memset0(thi[:])
        xcat_hi.append((thi, f"{pname}hi", cio))
        tlo = ptrans.tile([128, PH, PW], BF16, name=f"xlo{b4}")
        memset0(tlo[:])
        xcat_lo.append((tlo, f"{pname}lo", cio))
    # conv1 weights first in the queues so conv1(0) isn't starved: co=0 pair
    # on the SP queue, co=1 pair on the Act HWDGE queue
    w1s = []
    for co in range(2):
        q = nc.sync if co == 0 else nc.scalar
        w1h_sb = ptrans.tile([128, 4608], BF16, name=f"w1h_sb{co}")
        q.dma_start(w1h_sb[:], prm["w1Th"][:, co * 4608:(co + 1) * 4608])
        w1l_sb = ptrans.tile([128, 4608], BF16, name=f"w1l_sb{co}")
        q.dma_start(w1l_sb[:], prm["w1Tl"][:, co * 4608:(co + 1) * 4608])
        w1s.append((w1h_sb, w1l_sb))

    # strip-wise interior loads, strip-major so conv1(0) starts early;
    # hi tensors on the SP queue, lo tensors on the Act HWDGE queue
    for nt in range(8):
        rows = slice(nt * 8, nt * 8 + 8)
        prow = slice(nt * 8 + 1, nt * 8 + 9)
        for t, pname, cio in xcat_hi:
            nc.sync.dma_start(t[:, prow, 1:65],
                              prm[pname][cio * 128:(cio + 1) * 128, rows, :])
        for t, pname, cio in xcat_lo:
            nc.scalar.dma_start(t[:, prow, 1:65],
                                prm[pname][cio * 128:(cio + 1) * 128, rows, :])
    xcat_hi = [t for t, _, _ in xcat_hi]
    xcat_lo = [t for t, _, _ in xcat_lo]

    h_hi = []
    h_lo = []
    for co in range(2):
        t = ptrans.tile([128, PH, PW], BF16, name=f"h_hi{co}")
        memset0(t[:])
        h_hi.append(t)
        t = ptrans.tile([128, PH, PW], BF16, name=f"h_lo{co}")
        memset0(t[:])
        h_lo.append(t)

    # ---------------- grid-math persistent scratch ----------------
    G = [128, 32]

    def f32t(name):
        return pc.tile(G, F32, name=name)

    ixiy = pc.tile([128, 64], F32, name="ixiy")
    xi_i = pc.tile(G, I32, name="xi_i")
    yi_i = pc.tile(G, I32, name="yi_i")
    fx0 = f32t("fx0")
    fy0 = f32t("fy0")
    corr = f32t("corr")
    corr2 = f32t("corr2")
    wx = f32t("wx")
    wy = f32t("wy")
    vx0 = f32t("vx0")
    vx1 = f32t("vx1")
    vy0 = f32t("vy0")
    vy1 = f32t("vy1")
    va = f32t("va")
    vb = f32t("vb")
    xc1 = f32t("xc1")
    yc0 = f32t("yc0")
    yc1 = f32t("yc1")
    idxA_f = f32t("idxA_f")
    idxB_f = f32t("idxB_f")
    idxA = pc.tile(G, I32, name="idxA")
    idxB = pc.tile(G, I32, name="idxB")
    u = f32t("u")
    v = f32t("v")
    wxv = f32t("wxv")
    wyv = f32t("wyv")
    w00 = f32t("w00")
    w01 = f32t("w01")
    w10 = f32t("w10")
    w11 = f32t("w11")

    # ---------------- pipeline stage emitters ----------------
    def conv1_strip(nt):
        for co in range(2):
            w1h_sb, w1l_sb = w1s[co]
            ps = ppsum.tile([128, 512], F32, name="c1psum", tag="c1psum")
            first = True
            for t9 in range(9):
                dy, dx = t9 // 3 - 1, t9 % 3 - 1
                for ci in range(4):
                    col = (t9 * 4 + ci) * 128
                    rhs_hi = xcat_hi[ci][:, nt * 8 + 1 + dy:nt * 8 + 9 + dy,
                                         1 + dx:65 + dx]
                    rhs_lo = xcat_lo[ci][:, nt * 8 + 1 + dy:nt * 8 + 9 + dy,
                                         1 + dx:65 + dx]
                    last = (t9 == 8 and ci == 3)
                    nc.tensor.matmul(ps[:], w1h_sb[:, col:col + 128], rhs_hi,
                                     start=first, stop=False)
                    nc.tensor.matmul(ps[:], w1h_sb[:, col:col + 128], rhs_lo,
                                     start=False, stop=False)
                    nc.tensor.matmul(ps[:], w1l_sb[:, col:col + 128], rhs_hi,
                                     start=False, stop=last)
                    first = False
            # h_hi = relu(ps + b1) via ACT (bf16 write rounds);
            # h_lo = max(ps + b1, 0) - h_hi via one DVE op reading PSUM
            hiv = h_hi[co][:, nt * 8 + 1:nt * 8 + 9, 1:65]
            nc.scalar.activation(hiv, ps[:], ACT.Relu,
                                 bias=b1_sb[:, co:co + 1], scale=1.0)
            hstg = ptrans.tile([128, 512], F32, name="hstg", tag="hstg", bufs=2)
            nc.scalar.activation(hstg[:], ps[:], ACT.Relu,
                                 bias=b1_sb[:, co:co + 1], scale=1.0)
            nc.vector.tensor_sub(h_lo[co][:, nt * 8 + 1:nt * 8 + 9, 1:65],
                                 hstg[:], hiv)

    def conv2_strip(m):
        # psA rows = [w2h*h_hi (2) ; w2l*h_hi (2)], psB = w2h*h_lo. The
        # pair-sum across partitions happens post-transpose, in the free dim.
        # both accumulators packed into one PSUM bank: psA at partitions 0-3
        # (col group 0), psB at 32-33 (col group 1) -> they run concurrently
        ps6 = psmall.tile([34, 512], F32, name="c2ps", tag="c2ps")
        psA = ps6[0:4, :]
        psB = ps6[32:34, :]
        for t9 in range(9):
            dy, dx = t9 // 3 - 1, t9 % 3 - 1
            for ci in range(2):
                c = t9 * 2 + ci
                rhs_hi = h_hi[ci][:, m * 8 + 1 + dy:m * 8 + 9 + dy, 1 + dx:65 + dx]
                rhs_lo = h_lo[ci][:, m * 8 + 1 + dy:m * 8 + 9 + dy, 1 + dx:65 + dx]
                first = (c == 0)
                last = (c == 17)
                nc.tensor.matmul(psA, w2p_sb[:, c * 4:c * 4 + 4],
                                 rhs_hi, start=first, stop=last)
                nc.tensor.matmul(psB, w2p_sb[:, c * 4:c * 4 + 2],
                                 rhs_lo, start=first, stop=last,
                                 tile_position=(0, 32))
        offA = ptrans.tile([4, 512], F32, name="offA", tag="offA", bufs=2)
        offB = ptrans.tile([2, 512], F32, name="offB", tag="offB", bufs=2)
        nc.vector.tensor_copy(offA[:], psA)
        nc.vector.tensor_copy(offB[:], psB)
        return offA, offB

    def grid_strip(m, offAB):
        offA, offB = offAB
        # pixel-partition layout: pixel p = j*128 + i -> [i, j], j in [0,32)
        pst6 = ptpsum.tile([128, 4, 6], F32, name="offT_psum", tag="offT",
                           bufs=2)
        for jj in range(4):
            nc.tensor.transpose(pst6[:, jj, 0:4],
                                offA[:, jj * 128:(jj + 1) * 128], id_sb[:4, :4])
            nc.tensor.transpose(pst6[:, jj, 4:6],
                                offB[:, jj * 128:(jj + 1) * 128], id_sb[:2, :2])
        J = slice(4 * m, 4 * m + 4)        # j-cols of [128,32] tiles
        X = slice(8 * m, 8 * m + 8)        # cols of ixiy [128,64]
        i3 = ixiy[:, X].rearrange("p (j c) -> p j c", c=2)
        # ix/iy = 32*offset + base (scale folded into w2 on host; bxy = base)
        nc.vector.tensor_copy(i3, pst6[:, :, 0:2])
        nc.vector.tensor_add(i3, i3, pst6[:, :, 2:4])
        nc.vector.tensor_add(i3, i3, pst6[:, :, 4:6])
        nc.vector.tensor_add(ixiy[:, X], ixiy[:, X], bxy_sb[:, X])
        ix = ixiy[:, 8 * m:8 * m + 7:2]
        iy = ixiy[:, 8 * m + 1:8 * m + 8:2]

        # exact floor via int cast + correction
        nc.vector.tensor_copy(xi_i[:, J], ix)
        nc.vector.tensor_copy(fx0[:, J], xi_i[:, J])
        nc.vector.tensor_tensor(corr[:, J], fx0[:, J], ix, op=ALU.is_gt)
        nc.vector.tensor_sub(fx0[:, J], fx0[:, J], corr[:, J])
        nc.vector.tensor_copy(yi_i[:, J], iy)
        nc.vector.tensor_copy(fy0[:, J], yi_i[:, J])
        nc.vector.tensor_tensor(corr2[:, J], fy0[:, J], iy, op=ALU.is_gt)
        nc.vector.tensor_sub(fy0[:, J], fy0[:, J], corr2[:, J])

        nc.vector.tensor_sub(wx[:, J], ix, fx0[:, J])
        nc.vector.tensor_sub(wy[:, J], iy, fy0[:, J])

        def valid01(src, v0, v1):
            nc.vector.tensor_scalar(va[:, J], src[:, J], 0.0, None, op0=ALU.is_ge)
            nc.vector.tensor_scalar(vb[:, J], src[:, J], 63.0, None, op0=ALU.is_le)
            nc.vector.tensor_mul(v0[:, J], va[:, J], vb[:, J])
            nc.vector.tensor_scalar(va[:, J], src[:, J], -1.0, None, op0=ALU.is_ge)
            nc.vector.tensor_scalar(vb[:, J], src[:, J], 62.0, None, op0=ALU.is_le)
            nc.vector.tensor_mul(v1[:, J], va[:, J], vb[:, J])

        valid01(fx0, vx0, vx1)
        valid01(fy0, vy0, vy1)

        # clamped addresses (+1 guard-row shift folded into xc1)
        nc.vector.tensor_scalar(xc1[:, J], fx0[:, J], -1.0, 64.0,
                                op0=ALU.max, op1=ALU.min)
        nc.vector.tensor_scalar_add(xc1[:, J], xc1[:, J], 1.0)
        nc.vector.tensor_scalar(yc0[:, J], fy0[:, J], 0.0, 63.0,
                                op0=ALU.max, op1=ALU.min)
        nc.vector.tensor_scalar(yc1[:, J], fy0[:, J], 1.0, 0.0,
                                op0=ALU.add, op1=ALU.max)
        nc.vector.tensor_scalar_min(yc1[:, J], yc1[:, J], 63.0)

        nc.vector.scalar_tensor_tensor(idxA_f[:, J], yc0[:, J], 64.0, xc1[:, J],
                                       op0=ALU.mult, op1=ALU.add)
        nc.vector.scalar_tensor_tensor(idxB_f[:, J], yc1[:, J], 64.0, xc1[:, J],
                                       op0=ALU.mult, op1=ALU.add)
        nc.vector.tensor_copy(idxA[:, J], idxA_f[:, J])
        nc.vector.tensor_copy(idxB[:, J], idxB_f[:, J])

        # bilinear weights, validity folded in
        nc.vector.tensor_scalar(u[:, J], wx[:, J], -1.0, 1.0,
                                op0=ALU.mult, op1=ALU.add)
        nc.vector.tensor_mul(u[:, J], u[:, J], vx0[:, J])
        nc.vector.tensor_scalar(v[:, J], wy[:, J], -1.0, 1.0,
                                op0=ALU.mult, op1=ALU.add)
        nc.vector.tensor_mul(v[:, J], v[:, J], vy0[:, J])
        nc.vector.tensor_mul(wxv[:, J], wx[:, J], vx1[:, J])
        nc.vector.tensor_mul(wyv[:, J], wy[:, J], vy1[:, J])
        nc.vector.tensor_mul(w00[:, J], u[:, J], v[:, J])
        nc.vector.tensor_mul(w01[:, J], wxv[:, J], v[:, J])
        nc.vector.tensor_mul(w10[:, J], u[:, J], wyv[:, J])
        nc.vector.tensor_mul(w11[:, J], wxv[:, J], wyv[:, J])

        # issue the row gathers for this strip's 4 j-chunks now; the
        # bilinear combine happens one stage later
        gs = []
        for jj in range(4):
            j = 4 * m + jj
            gA = ptrans.tile([128, 512], BF16, name="gA", tag="gA", bufs=8)
            gB = ptrans.tile([128, 512], BF16, name="gB", tag="gB", bufs=8)
            nc.gpsimd.indirect_dma_start(
                out=gA[:], out_offset=None, in_=prm["xT2"][:],
                in_offset=bass.IndirectOffsetOnAxis(ap=idxA[:, j:j + 1], axis=0))
            nc.gpsimd.indirect_dma_start(
                out=gB[:], out_offset=None, in_=prm["xT2"][:],
                in_offset=bass.IndirectOffsetOnAxis(ap=idxB[:, j:j + 1], axis=0))
            gs.append((gA, gB))
        return gs

    def diff_strip(m, gs):
        xlw = []
        for co in range(2):
            t = ptrans.tile([128, 8, 64], F32, name=f"xlw{co}",
                            tag=f"xlw{co}", bufs=2)
            nc.sync.dma_start(t[:], prm["xl"][co * 128:(co + 1) * 128,
                                              m * 8:m * 8 + 8, :])
            xlw.append(t)
        for jj in range(4):
            j = 4 * m + jj
            gA, gB = gs[jj]
            acc = ptrans.tile([128, 256], F32, name="acc", tag="acc", bufs=2)
            nc.vector.tensor_scalar_mul(acc[:], gA[:, 0:256], w00[:, j:j + 1])
            nc.vector.scalar_tensor_tensor(acc[:], gA[:, 256:512],
                                           w01[:, j:j + 1], acc[:],
                                           op0=ALU.mult, op1=ALU.add)
            nc.vector.scalar_tensor_tensor(acc[:], gB[:, 0:256],
                                           w10[:, j:j + 1], acc[:],
                                           op0=ALU.mult, op1=ALU.add)
            nc.vector.scalar_tensor_tensor(acc[:], gB[:, 256:512],
                                           w11[:, j:j + 1], acc[:],
                                           op0=ALU.mult, op1=ALU.add)
            # transpose [128px, 256ch] to channel-major, diff = x_low - aligned
            for co in range(2):
                pt = ptpsum.tile([128, 128], F32, name="alT_psum", tag="alT")
                nc.tensor.transpose(pt[:], acc[:, co * 128:(co + 1) * 128],
                                    id_sb[:])
                nc.vector.tensor_sub(
                    diff_pad[co][:, 2 * j + 1:2 * j + 3, 1:65],
                    xlw[co][:, 2 * jj:2 * jj + 2, :], pt[:])

    # ---------------- front-end pipeline ----------------
    conv2_out = {}
    grid_out = {}
    for nt in range(11):
        if nt < 8:
            conv1_strip(nt)
        if 1 <= nt <= 8:
            conv2_out[nt - 1] = conv2_strip(nt - 1)
        if 2 <= nt <= 9:
            m = nt - 2
            grid_out[m] = grid_strip(m, conv2_out.pop(m))
        if nt >= 3:
            m = nt - 3
            diff_strip(m, grid_out.pop(m))

    ptrans.release()

    # ---------------- back end: 3x dynamic-scale conv branches ----------------
    pback = tc.alloc_tile_pool(name="pback", bufs=1)
    ctx_pools.append(pback)
    fused_pad = []
    for co in range(2):
        t = pback.tile([128, PH, PW], BF16, name=f"fused_pad{co}")
        memset0(t[:])
        fused_pad.append(t)

    # Software-pipelined: emit branch k+1's matmuls before branch k's SE
    # chain so the tiny SE matvecs (which wait on pooled stats) never stall
    # the PE queue between the big conv groups. xd / pooled tags use bufs=2
    # so branch k's data survives while branch k+1 computes.
    def emit_dw_matmuls(k):
        dwk_sb = pback.tile([128, 4608], BF16, name="dwk_sb", tag="dwk",
                            bufs=2)
        nc.scalar.dma_start(dwk_sb[:], prm[f"dwT{k}"][:])
        xd = []
        pooled_parts = []
        for co in range(2):
            xd_t = pback.tile([128, HW], BF16, name=f"xd{co}", tag=f"xd{co}",
                              bufs=2)
            xd.append(xd_t)
            pp_t = pc.tile([128, 8], F32, name=f"pooled_parts{co}",
                           tag=f"pooled_parts{co}", bufs=2)
            pooled_parts.append(pp_t)
        for co in range(2):
            for nt in range(8):
                ps = ppsum.tile([128, 512], F32, name="dwpsum", tag="c1psum")
                first = True
                for t9 in range(9):
                    dy, dx = t9 // 3 - 1, t9 % 3 - 1
                    for ci in range(2):
                        col = ((t9 * 2 + ci) * 2 + co) * 128
                        nc.tensor.matmul(
                            ps[:],
                            dwk_sb[:, col:col + 128],
                            diff_pad[ci][:, nt * 8 + 1 + dy:nt * 8 + 9 + dy,
                                         1 + dx:65 + dx],
                            start=first, stop=(t9 == 8 and ci == 1))
                        first = False
                nc.scalar.activation(
                    xd[co][:, nt * 512:(nt + 1) * 512], ps[:],
                    ACT.Identity, bias=db_sb[:, 2 * k + co:2 * k + co + 1],
                    scale=1.0, accum_out=pooled_parts[co][:, nt:nt + 1])
        return xd, pooled_parts

    def emit_se(k, xd, pooled_parts):
        # SE block (tiny matvecs); mean 1/HW folded into se1T on host
        pooled = []
        for co in range(2):
            p_t = pc.tile([128, 1], F32, name=f"pooled{co}", tag=f"pooled{co}")
            nc.vector.reduce_sum(p_t[:], pooled_parts[co][:],
                                 axis=mybir.AxisListType.X)
            pooled.append(p_t)
        pse = psmall.tile([128, 2], F32, name="pse", tag="pse")
        nc.tensor.matmul(pse[0:64, 0:1], se1_sb[:, k * 128:k * 128 + 64],
                         pooled[0][:], start=True, stop=False)
        nc.tensor.matmul(pse[0:64, 0:1], se1_sb[:, k * 128 + 64:k * 128 + 128],
                         pooled[1][:], start=False, stop=True)
        h1 = pc.tile([64, 1], F32, name="h1", tag="h1")
        nc.scalar.activation(h1[:], pse[0:64, 0:1], ACT.Relu,
                             bias=se1b_sb[:, k:k + 1], scale=1.0)
        for co in range(2):
            nc.tensor.matmul(pse[:, 1:2],
                             se2_sb[:, (k * 2 + co) * 128:(k * 2 + co + 1) * 128],
                             h1[:], start=True, stop=True)
            s_t = pc.tile([128, 1], F32, name=f"s{co}", tag=f"s{co}")
            nc.scalar.activation(s_t[:], pse[:, 1:2], ACT.Sigmoid,
                                 bias=se2b_sb[:, 2 * k + co:2 * k + co + 1],
                                 scale=1.0)
            # fused += xd * s
            if k == 0:
                nc.vector.tensor_scalar_mul(
                    fused_pad[co][:, 1:1 + H, 1:1 + W],
                    xd[co][:].rearrange("p (h w) -> p h w", h=H), s_t[:])
            else:
                nc.vector.scalar_tensor_tensor(
                    fused_pad[co][:, 1:1 + H, 1:1 + W],
                    xd[co][:].rearrange("p (h w) -> p h w", h=H), s_t[:],
                    fused_pad[co][:, 1:1 + H, 1:1 + W],
                    op0=ALU.mult, op1=ALU.add)

    prev = None
    for k in range(3):
        cur = emit_dw_matmuls(k)
        if prev is not None:
            emit_se(k - 1, *prev)
        prev = cur
    emit_se(2, *prev)

    # ---------------- attention + final ----------------
    for nt in range(8):
        attn = pback.tile([128, 512], BF16, name="attn", tag="attn", bufs=2)
        ps = ppsum.tile([128, 512], F32, name="sapsum", tag="c1psum")
        first = True
        for t9 in range(9):
            dy, dx = t9 // 3 - 1, t9 % 3 - 1
            for ci in range(2):
                col = (t9 * 2 + ci) * 128
                nc.tensor.matmul(
                    ps[:],
                    saw_sb[:, col:col + 128],
                    fused_pad[ci][:, nt * 8 + 1 + dy:nt * 8 + 9 + dy,
                                  1 + dx:65 + dx],
                    start=first, stop=(t9 == 8 and ci == 1))
                first = False
        nc.scalar.activation(attn[:], ps[:], ACT.Sigmoid, bias=sab_sb[:, 0:1],
                             scale=1.0)
        for co in range(2):
            xlt = pback.tile([128, 512], F32, name="xlt", tag="xlt", bufs=2)
            nc.sync.dma_start(
                xlt[:], prm["xl"][co * 128:(co + 1) * 128, nt * 8:(nt + 1) * 8, :])
            ot = pback.tile([128, 512], F32, name="ot", tag="ot", bufs=2)
            nc.vector.tensor_mul(
                ot[:], attn[:],
                diff_pad[co][:, nt * 8 + 1:nt * 8 + 9, 1:65])
            nc.vector.tensor_add(ot[:], ot[:], xlt[:])
            nc.sync.dma_start(
                prm["out"][co * 128:(co + 1) * 128, nt * 512:(nt + 1) * 512],
                ot[:])

    for p in reversed(ctx_pools):
        p.release()


def _build(repeat):
    nc = bacc.Bacc()
    prm = {}

    def din(name, shape, dt=F32):
        prm[name] = nc.declare_dram_parameter(name, list(shape), dt,
                                              isOutput=False)

    din("xl", [C, H, W])
    for nm in ["xlhi", "xllo", "xhhi", "xhlo"]:
        din(nm, [C, H, W], BF16)
    din("xT2", [TBL_ROWS, 512], BF16)
    din("w1Th", [128, 9216], BF16)
    din("w1Tl", [128, 9216], BF16)
    din("b1", [128, 2])
    din("w2p", [128, 72], BF16)
    din("bxy", [128, 64])
    din("ident", [128, 128])
    for k in range(3):
        din(f"dwT{k}", [128, 4608], BF16)
    din("db2", [128, 6])
    din("se1T", [128, 384])
    din("se1b2", [64, 3])
    din("se2T", [64, 768])
    din("se2b2", [128, 6])
    din("sawT", [128, 2304], BF16)
    din("sab_bc", [128, 1])
    prm["out"] = nc.declare_dram_parameter("out", [C, HW], F32, isOutput=True)

    with tile.TileContext(nc) as tc:
        pc_const, cn = _load_consts(nc, tc, prm)
        _emit_body(nc, tc, prm, cn, first_iter=True)
        if repeat > 1:
            with tc.For_i(0, repeat - 1, 1):
                _emit_body(nc, tc, prm, cn, first_iter=False)
        pc_const.release()
    nc.finalize()
    return nc


def _prep_inputs(x_low, x_high, a1w, a1b, bn_g, bn_b, bn_m, bn_v, a2w, a2b,
                 dw, db, se1w, se1b, se2w, se2b, saw, sab):
    """Host-side weight prep shared by all cores + per-core activation prep."""
    import ml_dtypes
    f32 = np.float32
    bf16 = ml_dtypes.bfloat16
    # conv1 with BN folded
    scale = (bn_g / np.sqrt(bn_v + EPS)).astype(f32)  # [256]
    w1f = (a1w * scale[:, None, None, None]).astype(f32)  # [256,512,3,3]
    b1f = ((a1b - bn_m) * scale + bn_b).astype(f32)  # [256]
    # host lhsT layout [k(128), ty,tx, ci(4), co(2), m(128)] -> [128, 9216]
    arr = w1f.reshape(2, 128, 4, 128, 3, 3)  # [co, m, ci, k, ty, tx]
    w1T = np.ascontiguousarray(arr.transpose(3, 0, 4, 5, 2, 1)).reshape(128, 9216)
    w1Th = w1T.astype(bf16)
    w1Tl = (w1T - w1Th.astype(np.float32)).astype(bf16)
    b1h = np.ascontiguousarray(b1f.reshape(2, 128).T)  # [128, 2]

    # conv2, grid scale W/2 = 32 folded in; hi/lo M-packed [w2h|w2l] per chunk
    w2f = (a2w * 32.0).astype(f32)  # [2, 256, 3, 3]
    arr = w2f.reshape(2, 2, 128, 3, 3)  # [m, ci, k, ty, tx]
    w2T = np.ascontiguousarray(arr.transpose(2, 3, 4, 1, 0))  # [k,ty,tx,ci,m]
    w2h = w2T.astype(bf16)
    w2l = (w2T - w2h.astype(np.float32)).astype(bf16)
    w2p = np.concatenate([w2h.reshape(128, 18, 2), w2l.reshape(128, 18, 2)],
                         axis=2)  # [k, chunk, 4]
    w2p = np.ascontiguousarray(w2p).reshape(128, 72)

    # base grid (+a2b*32): pixel p = j*128+i ; h=p//64, w=p%64
    lin = np.linspace(-1.0, 1.0, 64, dtype=f32)
    pidx = (np.arange(32)[None, :] * 128 + np.arange(128)[:, None])  # [128,32]
    bx = ((lin[pidx // 64] + 1.0) * 32.0 - 0.5 + 32.0 * f32(a2b[0])).astype(f32)
    by = ((lin[pidx % 64] + 1.0) * 32.0 - 0.5 + 32.0 * f32(a2b[1])).astype(f32)
    bxy = np.empty((128, 64), f32)
    bxy[:, 0::2] = bx
    bxy[:, 1::2] = by

    # diff convs (bf16)
    dwT = []
    for k in range(3):
        arr = dw[k].astype(f32).reshape(2, 128, 2, 128, 3, 3)  # [co,m,ci,kk,ty,tx]
        dwT.append(np.ascontiguousarray(
            arr.transpose(3, 4, 5, 2, 0, 1)).reshape(128, 4608).astype(bf16))
    db2 = np.ascontiguousarray(db.astype(f32).reshape(3, 2, 128).transpose(2, 0, 1)
                               ).reshape(128, 6)

    # SE (mean 1/HW folded into se1T)
    se1T = np.ascontiguousarray(
        (se1w.astype(f32) / HW).transpose(0, 2, 1).reshape(3, 2, 128, 64)
        .transpose(2, 0, 1, 3)).reshape(128, 384)
    se1b2 = np.ascontiguousarray(se1b.astype(f32).T)  # [64, 3]
    se2T = np.ascontiguousarray(
        se2w.astype(f32).transpose(0, 2, 1).reshape(3, 64, 2, 128)
        .transpose(1, 0, 2, 3)).reshape(64, 768)
    se2b2 = np.ascontiguousarray(se2b.astype(f32).reshape(3, 2, 128)
                                 .transpose(2, 0, 1)).reshape(128, 6)

    # attention conv, weights replicated to M=128 (bf16)
    arr = saw.astype(f32).reshape(1, 2, 128, 3, 3)  # [m=1, ci, k, ty, tx]
    arr = np.broadcast_to(arr, (128, 2, 128, 3, 3))  # replicate m
    sawT = np.ascontiguousarray(
        arr.transpose(2, 3, 4, 1, 0)).reshape(128, 2304).astype(bf16)
    sab_bc = np.full((128, 1), f32(sab[0]), f32)

    shared = dict(w1Th=w1Th, w1Tl=w1Tl, b1=b1h, w2p=w2p, bxy=bxy,
                  ident=np.eye(128, dtype=f32),
                  dwT0=dwT[0], dwT1=dwT[1], dwT2=dwT[2], db2=db2,
                  se1T=se1T, se1b2=se1b2, se2T=se2T, se2b2=se2b2,
                  sawT=sawT, sab_bc=sab_bc)

    in_maps = []
    for b in range(B):
        xlb = np.ascontiguousarray(x_low[b].astype(f32))
        xhb = x_high[b].astype(f32)
        xlhi = xlb.astype(bf16)
        xllo = (xlb - xlhi.astype(f32)).astype(bf16)
        xhhi = xhb.astype(bf16)
        xhlo = (xhb - xhhi.astype(f32)).astype(bf16)
        XT = np.ascontiguousarray(xhb.reshape(C, HW).T)  # [4096, 256]
        XT2 = np.zeros((TBL_ROWS, 512), f32)
        XT2[1:1 + HW, :256] = XT
        XT2[0:HW, 256:] = XT
        m = dict(shared)
        m["xl"] = xlb
        m["xlhi"] = np.ascontiguousarray(xlhi)
        m["xllo"] = np.ascontiguousarray(xllo)
        m["xhhi"] = np.ascontiguousarray(xhhi)
        m["xhlo"] = np.ascontiguousarray(xhlo)
        m["xT2"] = XT2.astype(bf16)
        in_maps.append(m)
    return in_maps


_last_results = None


def kernel(**inputs):
    global _last_results
    repeat = int(os.environ.get("KERNEL_REPEAT", "1"))
    if repeat not in _nc_cache:
        _nc_cache[repeat] = _build(repeat)
    nc = _nc_cache[repeat]
    in_maps = _prep_inputs(**inputs)
    res = run_bass_kernel_spmd(nc, in_maps, list(range(NCORES)))
    _last_results = res
    out = np.stack([res.results[b]["out"].reshape(C, H, W) for b in range(B)])
    return out.astype(np.float32)


if __name__ == "__main__":
    import reference
    inputs = {k: np.asarray(v) for k, v in reference.setup_inputs().items()}
    expected = np.asarray(reference.reference(**inputs))
    actual = kernel(**inputs)
    err = np.abs(actual - expected).max()
    rel = err / np.abs(expected).max()
    print(f"abs err: {err:.4e}  rel err: {rel:.4e}")
